# revision 6
# baseline (speedup 1.0000x reference)
"""Causal self-attention (B=4, N=2048, D=1024, H=16) on 8 Trainium2 NeuronCores.

Sharding: 8 cores = 4 batches x 2 head-groups (tensor-parallel over heads,
data-parallel over batch). Each core runs one SPMD Bass kernel computing, for
its (batch b, head-group g of 8 heads):

  - q/k projections (bf16 matmuls, fp32 PSUM accumulation), bias added on the
    psum->sbuf copy; v projection with a constant ones-column appended per head
  - causal attention per head: scores^T tiles in S^T layout ([k part, q free])
    via TensorE, exp on ScalarE with the 1/sqrt(HD) scale folded in, diagonal
    causal mask via a bf16 multiply
  - AV flipped: attexp^T chunks are the STATIONARY operand, [v | 1] the moving
    one, so o_ext accumulates as [q=128, 65] in PSUM using all 128 output
    partitions (2x fewer PE columns than the [65, q] orientation) and row-sums
    land in column 64 for free. Four q-chunks pack into one PSUM bank with a
    single accumulation-group start (start=True marks the whole zero region
    pending, each chunk's first write then overwrites).
  - softmax normalization is now per-PARTITION: reciprocal of the packed
    denominator column + one stride-0-broadcast multiply on VectorE (no DRAM
    bounce), writing sa as [q, feat] bf16
  - sa pair-tiles [128q, 128feat] are transposed back to [feat, n] on TensorE
    (identity matmul), v-bias folded into the psum->sbuf copy per partition
  - the partial output projection out_p = sa_g @ Wproj[:, g-cols].T -> [N, D]

The host casts x to bf16 (the kernel loads xt directly, no staging casts) and
sums the two head-group partials per batch plus bproj.
"""
import os

import numpy as np
import ml_dtypes
import bass_rust

import concourse.bass as bass
import concourse.mybir as mybir
import concourse.tile as tile_mod

from concourse.tile import TileContext
from concourse.vector_clock import ScopedClock
from concourse.bass_utils import run_bass_kernel_spmd

F32 = mybir.dt.float32
BF16 = mybir.dt.bfloat16
AF = mybir.ActivationFunctionType

B = 4         # batch
N = 2048      # sequence length
D = 1024      # model dim
HD = 64       # head dim
HLOC = 8      # heads per core
NPAIR = 4     # head pairs per core
DSUB = 8      # D / 128 contraction subtiles
NCH = 16      # N / 128 chunks
QC = 4        # N / 512 q-chunks
VW = HLOC * (HD + 1)  # v storage width per 128-chunk (65 per head)


def _patch_tile_drain():
    """The walrus build in this container rejects sync waits attached to an SP
    Drain (setupSyncWait<CTRL_NO_STRUCT>); emit one wait_ge per semaphore
    before a bare drain instead."""
    if getattr(tile_mod.TileContext, "_drain_patched", False):
        return

    def _drain_and_barrier(self, tick_clock, wait_clock):
        probe = mybir.InstNoOp(name="I-drainprobe", ins=[], outs=[])
        probe.engine = mybir.EngineType.SP
        wait_clock.add_sem_waits(probe, ScopedClock({None: tick_clock.global_clock}))
        sem_by_num = {h.num: h for h in self.sems.allocated().values()}
        for w in (probe.sync_info.on_wait if probe.sync_info else []):
            self.nc.sync.wait_ge(sem_by_num[w.id], w.wait_value)
        self.nc.sync.drain()
        self.nc.all_engine_barrier()
        popped = self.nc._tile_sem_poison_stack.pop()
        assert popped is self._sem_poison
        self.nc.clear_and_free_semaphores(list(self.sems.allocated().values()))
        self.nc.all_engine_barrier()

    tile_mod.TileContext._drain_and_barrier = _drain_and_barrier
    tile_mod.TileContext._drain_patched = True


def _split_excess_waits(nc, max_waits=1):
    """This walrus accepts at most one sync wait per instruction; hoist the
    rest onto standalone EventSemaphore waits on the same engine stream
    (waits fire at issue time, so ordering semantics are identical)."""
    idx = 0
    for fn in nc.m.functions:
        for blk in fn.blocks:
            out = []
            for inst in blk.instructions:
                si = inst.sync_info
                waits = list(si.on_wait) if (si and si.on_wait) else []
                if len(waits) > max_waits:
                    for w in waits[:-max_waits]:
                        ev = mybir.InstEventSemaphore(
                            name=f"I-wsplit{idx}", ins=[], outs=[])
                        idx += 1
                        ev.engine = inst.engine
                        ev.sync_info = bass_rust.SyncInfo(on_wait=[w],
                                                          on_update=[])
                        nc.register_instruction(ev, overwrite=True)
                        out.append(ev)
                    si.on_wait = waits[-max_waits:]
                out.append(inst)
            blk.instructions = out


def build_kernel():
    _patch_tile_drain()
    nc = bass.Bass("TRN2")

    xt = nc.dram_tensor("xt", [DSUB, 128, N], BF16, kind="ExternalInput")
    wqk = nc.dram_tensor("wqk", [128, NPAIR, 2, DSUB, 128], BF16, kind="ExternalInput")
    wv = nc.dram_tensor("wv", [128, DSUB, HLOC * HD], BF16, kind="ExternalInput")
    wproj = nc.dram_tensor("wproj", [128, 4, D], BF16, kind="ExternalInput")
    bqk = nc.dram_tensor("bqk", [128, NPAIR * 2], F32, kind="ExternalInput")
    bv = nc.dram_tensor("bv", [128, NPAIR], F32, kind="ExternalInput")
    out_p = nc.dram_tensor("out_p", [N, D], F32, kind="ExternalOutput")

    # causal keep-mask for the diagonal 128x128 block of S^T: keep q >= k
    mask_np = np.triu(np.ones((128, 128), np.float32)).astype(ml_dtypes.bfloat16)
    maskt = nc.inline_tensor(mask_np, name="diagmask")
    ident_np = np.eye(128, dtype=ml_dtypes.bfloat16)
    identt = nc.inline_tensor(ident_np, name="ident")

    with TileContext(nc) as tc:
        with (
            tc.tile_pool(name="persist", bufs=1) as persist,
            tc.tile_pool(name="qk", bufs=2) as qkpool,
            tc.tile_pool(name="ae", bufs=6) as aepool,
            tc.tile_pool(name="norm", bufs=8) as normpool,
            tc.tile_pool(name="outst", bufs=4) as outpool,
            tc.tile_pool(name="acc512", bufs=2, space="PSUM") as acc512,
            tc.tile_pool(name="scps", bufs=2, space="PSUM") as scps,
            tc.tile_pool(name="oax", bufs=2, space="PSUM") as oax,
        ):
            # ---- persistent SBUF tensors ----
            xtb = persist.tile([128, DSUB, N], BF16, tag="xtb")
            vsb = persist.tile([128, NCH, VW], BF16, tag="vsb")
            saq = persist.tile([128, NPAIR, NCH, 128], BF16, tag="saq")
            sasbT = persist.tile([128, NPAIR, N], BF16, tag="sasbT")
            wvsb = persist.tile([128, DSUB, HLOC * HD], BF16, tag="wvsb")
            wprojsb = persist.tile([128, 4, D], BF16, tag="wprojsb")
            wqksb = persist.tile([128, NPAIR, 2, DSUB, 128], BF16, tag="wqksb")
            bqksb = persist.tile([128, NPAIR * 2], F32, tag="bqksb")
            bvsb = persist.tile([128, NPAIR], F32, tag="bvsb")
            masksb = persist.tile([128, 128], BF16, tag="masksb")
            idsb = persist.tile([128, 128], BF16, tag="idsb")

            # ---- phase A0: PE warm-up ----
            # Dummy matmuls on a memset tile run during the initial DMA wait,
            # releasing the HAM clock gate so the first real matmuls issue at
            # full rate.
            warm = persist.tile([128, 128], BF16, tag="warm")
            nc.vector.memset(warm[:], 0.0)
            wps = scps.tile([128, 1024], F32, tag="sc", name="warmps")
            for i in range(48):
                nc.tensor.matmul(wps[:, 0:128], lhsT=warm[:], rhs=warm[:],
                                 start=True, stop=True)

            # ---- phase A: loads ----
            # x (PE's critical input) streams on the SP queue in (nq, s)
            # quarters so phase B starts as soon as the first quarter lands;
            # weights go down the ScalarE DMA queue in parallel.
            nc.scalar.dma_start(wvsb[:, 0:1, :], wv[:, 0:1, :])
            for nq in range(4):
                for s in range(DSUB):
                    nc.sync.dma_start(xtb[:, s, nq * 512:(nq + 1) * 512],
                                      xt[s, :, nq * 512:(nq + 1) * 512])
                    if nq == 0 and s == 0:
                        nc.scalar.dma_start(wvsb[:, 1:DSUB, :], wv[:, 1:DSUB, :])
            for p in range(NPAIR):  # per-pair so C1(p=0) unblocks early
                nc.scalar.dma_start(wqksb[:, p], wqk[:, p])
            nc.sync.dma_start(masksb[:], maskt[:])
            nc.sync.dma_start(idsb[:], identt[:])
            nc.sync.dma_start(bqksb[:], bqk[:])
            nc.sync.dma_start(bvsb[:], bv[:])
            nc.sync.dma_start(wprojsb[:], wproj[:])

            # ------------------------------------------------------------
            # Emission helpers.  The attention window is Act(exp)-bound in
            # stretches; a FIFO filler queue interleaves deferred PE work
            # (B chunks, next pair's C1 tiles, transposes, D chunks) into
            # the score/AV stream, driven by virtual pe/act clocks.
            # ------------------------------------------------------------
            vview = vsb[:].rearrange("p c (h e) -> p c h e", e=HD + 1)
            clock = {"pe": 0.0, "act": 0.0}

            def emit_b(nch):
                ps = acc512.tile([128, 512], F32, tag="acc",
                                 name=f"bps_{nch}")
                for s in range(DSUB):
                    nc.tensor.matmul(
                        ps[:],
                        lhsT=xtb[:, s, nch * 128:(nch + 1) * 128],
                        rhs=wvsb[:, s, :],
                        start=(s == 0), stop=(s == DSUB - 1),
                    )
                nc.vector.tensor_copy(
                    vview[:, nch, :, 0:HD],
                    ps[:].rearrange("p (h e) -> p h e", e=HD),
                )
                nc.vector.memset(vview[:, nch, :, HD:HD + 1], 1.0)
                clock["pe"] += 1707

            def emit_c1(p, qc, w, stacks):
                ps = acc512.tile([128, 512], F32, tag="acc",
                                 name=f"c1ps_{p}_{qc}_{w}")
                for s in range(DSUB):
                    nc.tensor.matmul(
                        ps[:],
                        lhsT=wqksb[:, p, w, s, :],
                        rhs=xtb[:, s, qc * 512:(qc + 1) * 512],
                        start=(s == 0), stop=(s == DSUB - 1),
                    )
                nc.vector.tensor_scalar_add(
                    stacks[w][:, qc * 512:(qc + 1) * 512], ps[:],
                    bqksb[:, p * 2 + w:p * 2 + w + 1],
                )
                clock["pe"] += 1707

            def emit_t(p, qt):
                tps = scps.tile([128, 128], BF16, tag="sc",
                                name=f"tp_{p}_{qt}")
                nc.tensor.transpose(tps[:], saq[:, p, qt, :], idsb[:])
                nc.vector.tensor_scalar_add(
                    sasbT[:, p, qt * 128:(qt + 1) * 128], tps[:],
                    bvsb[:, p:p + 1],
                )
                clock["pe"] += 53

            def emit_d(nch, dc):
                ps = acc512.tile([128, 512], F32, tag="acc",
                                 name=f"dps_{nch}_{dc}")
                for j in range(4):
                    nc.tensor.matmul(
                        ps[:],
                        lhsT=sasbT[:, j, nch * 128:(nch + 1) * 128],
                        rhs=wprojsb[:, j, dc * 512:(dc + 1) * 512],
                        start=(j == 0), stop=(j == 3),
                    )
                ob = outpool.tile([128, 512], F32, tag="ob")
                nc.vector.tensor_copy(ob[:], ps[:])
                oeng = nc.sync if (nch + dc) % 2 == 0 else nc.gpsimd
                oeng.dma_start(
                    out_p[nch * 128:(nch + 1) * 128, dc * 512:(dc + 1) * 512],
                    ob[:],
                )
                clock["pe"] += 853

            import collections as _c
            fillq = _c.deque()  # (kind, key, cost, closure)

            def pop_fillers(slack=0.0):
                while fillq and clock["pe"] + slack < clock["act"]:
                    fillq.popleft()[3]()

            def drain(pred):
                keep = _c.deque()
                while fillq:
                    item = fillq.popleft()
                    if pred(item):
                        item[3]()
                    else:
                        keep.append(item)
                fillq.extend(keep)

            # ---- phase C: per head-pair q/k projection + attention ----
            all_stacks = {}
            for p in range(NPAIR):
                all_stacks[p] = [qkpool.tile([128, N], BF16, tag=f"qk{w}",
                                             name=f"qk{w}_{p}")
                                 for w in range(2)]

            # prologue: just enough for the first exp to start — q/k for the
            # first q-half of pair 0, v for k-blocks 0..3
            for qc in (0, 1):
                for w in range(2):
                    emit_c1(0, qc, w, all_stacks[0])
            for nch in range(4):
                emit_b(nch)

            b_next = [4]  # next unemitted B chunk

            for p in range(NPAIR):
                stacks = all_stacks[p]
                if p == 0:
                    # rest of v + rest of p0's C1 go to the filler queue
                    for nch in range(4, 8):
                        fillq.append(("b", nch, 1707,
                                      lambda nch=nch: emit_b(nch)))
                    for qc in (2, 3):
                        for w in range(2):
                            fillq.append(("c1", (0, qc),  1707,
                                          lambda qc=qc, w=w, st=stacks:
                                          emit_c1(0, qc, w, st)))
                    for nch in range(8, NCH):
                        fillq.append(("b", nch, 1707,
                                      lambda nch=nch: emit_b(nch)))
                qst, kst = stacks

                # C2: attention, half-outer so both heads complete each
                # q-half together (transposes/D can then chase).  Score/exp
                # pipelined one k-block ahead of AV.
                drain(lambda it: it[0] == "c1" and it[1][0] == p
                      and it[1][1] <= 1)
                for half in range(2):
                    if half == 1:
                        # half1 scores need this pair's qc2/qc3 projections
                        drain(lambda it: it[0] == "c1" and it[1][0] == p)
                    for e in range(2):
                        h = 2 * p + e
                        q_ap = qst[64 * e:64 * e + 64]
                        k_ap = kst[64 * e:64 * e + 64]
                        o_ps = [oax.tile([128, 4, HD + 1], F32, tag="o",
                                         name=f"o_{h}_{half}_{qq}")
                                for qq in range(2)]
                        started = [False, False]

                        def emit_norm(qq, h=h, half=half, e=e, p=p,
                                      o_ps=o_ps):
                            recip = normpool.tile([128, 4], F32, tag="recip",
                                                  name=f"rc_{h}_{half}_{qq}")
                            nc.vector.reciprocal(recip[:], o_ps[qq][:, :, HD])
                            rbc = bass.AP(
                                tensor=recip.tensor, offset=recip.offset,
                                ap=list(recip.ap[:2]) + [[0, HD]],
                            )
                            cb = 8 * half + 4 * qq
                            nc.vector.tensor_mul(
                                saq[:, p, cb:cb + 4, 64 * e:64 * e + 64],
                                o_ps[qq][:, :, 0:HD], rbc,
                            )

                        def emit_scores(t, h=h, half=half, q_ap=q_ap,
                                        k_ap=k_ap):
                            pstart = max(128 * t, 1024 * half)
                            wp = 1024 * half + 1024 - pstart
                            sc = scps.tile([128, 1024], F32, tag="sc")
                            off = 0
                            while off < wp:
                                mv = min(512, wp - off)
                                nc.tensor.matmul(
                                    sc[:, off:off + mv],
                                    lhsT=k_ap[:, 128 * t:128 * t + 128],
                                    rhs=q_ap[:, pstart + off:pstart + off + mv],
                                    start=True, stop=True,
                                )
                                off += 512
                            clock["pe"] += wp * 0.42
                            ae = aepool.tile([128, 1024], BF16, tag="ae")
                            nc.scalar.activation(ae[:, :wp], sc[:, :wp],
                                                 AF.Exp, scale=0.125)
                            clock["act"] = max(clock["act"],
                                               clock["pe"] + 150) + wp * 0.833 + 185
                            if pstart == 128 * t:
                                nc.vector.tensor_mul(ae[:, 0:128], ae[:, 0:128],
                                                     masksb[:])
                            return ae, pstart, clock["act"]

                        def emit_av(t, ae, pstart, expdone, h=h, half=half,
                                    o_ps=o_ps, started=started):
                            # vsb[t] must exist before AV reads it
                            while b_next[0] <= t:
                                drain(lambda it: it[0] == "b"
                                      and it[1] == b_next[0])
                                b_next[0] += 1
                            clock["pe"] = max(clock["pe"], expdone)
                            for g in range(max(t, 8 * half), 8 * half + 8):
                                qq, j = (g - 8 * half) // 4, (g - 8 * half) % 4
                                qoff = 128 * g - pstart
                                is_last = (t == 8 * half + 4 * qq + 3
                                           and j == 3)
                                nc.tensor.matmul(
                                    o_ps[qq][:, j, :],
                                    lhsT=ae[:, qoff:qoff + 128],
                                    rhs=vsb[:, t,
                                            h * (HD + 1):(h + 1) * (HD + 1)],
                                    start=(not started[qq]), stop=is_last,
                                    skip_group_check=True,
                                )
                                started[qq] = True
                                clock["pe"] += 28
                            for qq in range(2):
                                if t == 8 * half + 4 * qq + 3:
                                    emit_norm(qq)

                        T = 8 * half + 8
                        prev = None
                        for t in range(T):
                            cur = (t,) + emit_scores(t)[0:3]
                            if prev is not None:
                                pop_fillers()
                                emit_av(*prev)
                            prev = cur
                        pop_fillers()
                        emit_av(*prev)

                    # both heads done with this half: transposes can chase,
                    # and after the last pair also the D chunks
                    qt0 = 8 * half
                    for qt in range(qt0, qt0 + 8):
                        fillq.append(("t", (p, qt), 53,
                                      lambda p=p, qt=qt: emit_t(p, qt)))
                    if p == NPAIR - 1:
                        for nch in range(qt0, qt0 + 8):
                            for dc in range(2):
                                fillq.append(("d", (nch, dc), 853,
                                              lambda nch=nch, dc=dc:
                                              emit_d(nch, dc)))
                # next pair's C1 tiles become filler during this pair's C2
                if p + 1 < NPAIR:
                    for qc in range(QC):
                        for w in range(2):
                            fillq.append(("c1", (p + 1, qc), 1707,
                                          lambda p=p, qc=qc, w=w:
                                          emit_c1(p + 1, qc, w,
                                                  all_stacks[p + 1])))

            # ---- epilogue: whatever filler work remains ----
            drain(lambda it: True)

    _split_excess_waits(nc)
    return nc


# ---------------- host-side sharding ----------------

def prep_core_inputs(x, Wkqv, bkqv, Wproj, b, g):
    """Per-core input dict for core (batch b, head-group g)."""
    bf = ml_dtypes.bfloat16
    Wg = Wkqv[g * HLOC:(g + 1) * HLOC]         # [8, 192, 1024]
    bg = bkqv[g * HLOC:(g + 1) * HLOC]         # [8, 192]
    Wk, Wq, Wv = Wg[:, :HD], Wg[:, HD:2 * HD], Wg[:, 2 * HD:]
    bk, bq, bvv = bg[:, :HD], bg[:, HD:2 * HD], bg[:, 2 * HD:]

    xT = np.ascontiguousarray(x[b].T).reshape(DSUB, 128, N).astype(bf)

    # wqk[d, p, w, s, (e j)] = W(w)[2p+e, j, 128s+d]
    def stack_pairs(W):  # W [8, 64, 1024] -> [128, 4, 8, 128]
        t = W.reshape(NPAIR, 2, HD, DSUB, 128)               # p e j s d
        return t.transpose(4, 0, 3, 1, 2).reshape(128, NPAIR, DSUB, 128)

    wqk = np.ascontiguousarray(
        np.stack([stack_pairs(Wq), stack_pairs(Wk)], axis=2)  # [128, 4, 2, 8, 128]
    ).astype(bf)
    wv = np.ascontiguousarray(
        Wv.reshape(HLOC, HD, DSUB, 128).transpose(3, 2, 0, 1).reshape(128, DSUB, HLOC * HD)
    ).astype(bf)
    wproj = np.ascontiguousarray(
        Wproj.T[g * 512:(g + 1) * 512].reshape(4, 128, D).transpose(1, 0, 2)
    ).astype(bf)

    bqk = np.zeros((128, NPAIR * 2), np.float32)
    bvh = np.zeros((128, NPAIR), np.float32)
    for p in range(NPAIR):
        for e in range(2):
            h = 2 * p + e
            bqk[64 * e:64 * e + 64, 2 * p + 0] = bq[h]
            bqk[64 * e:64 * e + 64, 2 * p + 1] = bk[h]
            bvh[64 * e:64 * e + 64, p] = bvv[h]

    return {"xt": xT, "wqk": wqk, "wv": wv, "wproj": wproj,
            "bqk": bqk, "bv": bvh}


_NC_CACHE = {}


def _get_nc():
    if "nc" not in _NC_CACHE:
        _NC_CACHE["nc"] = build_kernel()
    return _NC_CACHE["nc"]


def kernel(x, Wkqv, bkqv, Wproj, bproj):
    x = np.asarray(x, np.float32)
    Wkqv = np.asarray(Wkqv, np.float32)
    bkqv = np.asarray(bkqv, np.float32)
    Wproj = np.asarray(Wproj, np.float32)
    bproj = np.asarray(bproj, np.float32)

    try:  # tracing needs the axon NTFF hook, absent in this container
        from antenv.axon_hooks import get_axon_ntff_profile_hook  # noqa: F401
    except ImportError:
        os.environ.setdefault("BASS_NEVER_TRACE", "1")

    in_maps = [prep_core_inputs(x, Wkqv, bkqv, Wproj, b, g)
               for b in range(B) for g in range(2)]
    nc = _get_nc()
    res = run_bass_kernel_spmd(nc, in_maps, core_ids=list(range(8)))
    parts = [res.results[i]["out_p"] for i in range(8)]

    out = np.empty((B, N, D), np.float32)
    for b in range(B):
        out[b] = parts[2 * b] + parts[2 * b + 1] + bproj[None, :]
    return out
